# revision 12
# baseline (speedup 1.0000x reference)
"""Trainium2 Bass kernel for nn_BoneRefusion (17-group BoneMLP over [B,T,16,3]).

Data parallel over batch (8 cores). Per core, tokens are packed in pairs
(2 "sets" of S = TC/2 tokens each) so the layer-1 contraction uses K=97
(48 features set A, ones row, 48 features set B).

Per 512-pair block:
  - L1: 4 full matmuls [97,128]x[97,512] -> hp0..hp3 (one PSUM bank each).
    Stationary is block-diagonal over sets; the ones row bakes in b1.
    Group 16 (the 17th group, 16 hidden x 2 sets = M 32) is handled by a
    column-tiled matmul into a shared PSUM bank, one 32-row strip per
    block of a 4-block superstep.
  - ReLU evacuations PSUM->SBUF bf16: hp0/hp1 on ScalarE, hp2/hp3 on
    VectorE (both engines run ~1x on fp32 PSUM reads, so the split is
    the bottleneck balance of the kernel).
  - L2: 4 column-tiled concurrent matmuls (tile_position strips) into a
    single PSUM bank [128, 512]: strip w rows 32w+0:12 = set-A outs of
    groups 4w..4w+3, rows 32w+12:24 = set-B outs. Group-16's L2 runs
    once per superstep as a block-diagonal [128,32] matmul over the
    shared g16 h bank.
  - Output evacuation adds b2 (ScalarE Identity+bias) and casts to bf16;
    the b2-evac for block i is issued one block late so ScalarE never
    stalls waiting for L2.

Outputs leave the device feature-major bf16; the host transposes back.
"""

import sys

import numpy as np
import ml_dtypes

sys.path.insert(0, "/opt/trn_rl_repo")

import concourse.bass as bass
import concourse.mybir as mybir
import concourse.tile as tile
from concourse import bacc
from concourse.bass_utils import run_bass_kernel_spmd

BF16 = mybir.dt.bfloat16
F32 = mybir.dt.float32
BF16_NP = ml_dtypes.bfloat16

NG = 17          # groups
HID = 16         # hidden per group
B, T, NJ, C = 2048, 243, 16, 3
NF = NJ * C      # 48 input features per token
NCORES = 8
BC = B // NCORES           # batches per core
TC = BC * T                # tokens per core
S = TC // 2                # token pairs per core (2-set packing)
KX = 2 * NF + 2            # 98: 2x48 features + ones row + zero pad (even partition count spreads DMA descriptors)
NBLK = 512                 # token-pairs per block (one psum bank)
NFULL = S // NBLK          # 60 full blocks
TAILN = S - NFULL * NBLK   # 384
NSUPER = NFULL // 4        # 15 full supersteps
OG16_COLS = NSUPER * NBLK + TAILN  # 8064


def _host_weights(W1, b1, W2, b2, idx):
    """Build stationary operands + bias vectors on the host."""
    W1 = np.asarray(W1, np.float32)
    b1 = np.asarray(b1, np.float32)
    W2 = np.asarray(W2, np.float32)
    b2 = np.asarray(b2, np.float32)
    idx = np.asarray(idx)

    # Scatter per-group [12, 16] W1 blocks into the 48-feature space.
    # Padded limb rows of W1 are already zero, so += handles duplicates.
    w1full = np.zeros((NF, NG * HID), np.float32)
    for g in range(NG):
        for j in range(4):
            r = int(idx[g, j]) * C
            w1full[r:r + C, g * HID:(g + 1) * HID] += W1[g, j * C:(j + 1) * C, :]
    b1flat = b1.reshape(NG * HID)

    # Layer-1 stationary [97, 4, 128]: pass w covers groups 4w..4w+3,
    # set A in cols 0:64 (input rows 0:48), set B in cols 64:128 (rows
    # 49:97). Row 48 is the shared ones row carrying b1 for both sets.
    w1l = np.zeros((KX, 4, 128), np.float32)
    for w in range(4):
        blk = w1full[:, 64 * w:64 * w + 64]
        bias = b1flat[64 * w:64 * w + 64]
        w1l[0:NF, w, 0:64] = blk
        w1l[NF, w, 0:64] = bias
        w1l[NF + 1:2 * NF + 1, w, 64:128] = blk
        w1l[NF, w, 64:128] = bias

    # Group-16 L1 stationary [97, 32]: cols 0:16 set A, 16:32 set B.
    w1g = np.zeros((KX, 32), np.float32)
    w1g[0:NF, 0:16] = w1full[:, 256:272]
    w1g[NF, 0:16] = b1flat[256:272]
    w1g[NF + 1:2 * NF + 1, 16:32] = w1full[:, 256:272]
    w1g[NF, 16:32] = b1flat[256:272]

    # Layer-2 stationary [128, 4, 32]: pass w, h rows 0:64 = set A
    # (16 per group), 64:128 = set B; out cols 3j+c (A), 12+3j+c (B).
    w2l = np.zeros((128, 4, 32), np.float32)
    for w in range(4):
        for j in range(4):
            g = 4 * w + j
            w2l[16 * j:16 * j + 16, w, 3 * j:3 * j + 3] = W2[g]
            w2l[64 + 16 * j:64 + 16 * j + 16, w, 12 + 3 * j:12 + 3 * j + 3] = W2[g]

    # Group-16 L2 stationary [128, 32], block-diagonal over the 4 blocks
    # of a superstep: h rows 32b+0:16 (A) -> cols 6b+0:3, rows
    # 32b+16:32 (B) -> cols 6b+3:6.
    w2g = np.zeros((128, 32), np.float32)
    for bq in range(4):
        w2g[32 * bq:32 * bq + 16, 6 * bq:6 * bq + 3] = W2[16]
        w2g[32 * bq + 16:32 * bq + 32, 6 * bq + 3:6 * bq + 6] = W2[16]

    # Output biases per psum partition.
    b2v = np.zeros((128, 1), np.float32)
    for w in range(4):
        v = b2[4 * w:4 * w + 4].reshape(12)
        b2v[32 * w:32 * w + 12, 0] = v
        b2v[32 * w + 12:32 * w + 24, 0] = v
    b2g = np.zeros((32, 1), np.float32)
    b2g[0:24, 0] = np.tile(b2[16], 8)

    return (w1l.reshape(KX, 512).astype(BF16_NP), w1g.astype(BF16_NP),
            w2l.reshape(128, 128).astype(BF16_NP), w2g.astype(BF16_NP),
            b2v, b2g)


def _build_nc():
    nc = bacc.Bacc(
        "TRN2", target_bir_lowering=False, debug=False, num_devices=NCORES,
    )
    x2 = nc.dram_tensor("x2", [KX, S], BF16, kind="ExternalInput").ap()
    w1 = nc.dram_tensor("w1", [KX, 512], BF16, kind="ExternalInput").ap()
    w1g = nc.dram_tensor("w1g", [KX, 32], BF16, kind="ExternalInput").ap()
    w2 = nc.dram_tensor("w2", [128, 128], BF16, kind="ExternalInput").ap()
    w2g = nc.dram_tensor("w2g", [128, 32], BF16, kind="ExternalInput").ap()
    b2v = nc.dram_tensor("b2v", [128, 1], F32, kind="ExternalInput").ap()
    b2g = nc.dram_tensor("b2g", [32, 1], F32, kind="ExternalInput").ap()
    outd = nc.dram_tensor("outd", [120, S], BF16, kind="ExternalOutput").ap()
    og16d = nc.dram_tensor("og16d", [24, OG16_COLS], BF16,
                           kind="ExternalOutput").ap()

    RELU = mybir.ActivationFunctionType.Relu
    IDENT = mybir.ActivationFunctionType.Identity
    MAX = mybir.AluOpType.max
    ADD = mybir.AluOpType.add

    with tile.TileContext(nc) as tc:
        with (
            tc.tile_pool(name="singles", bufs=1) as singles,
            tc.tile_pool(name="xin", bufs=6) as xin,
            tc.tile_pool(name="hsb", bufs=2) as hsb,
            tc.tile_pool(name="osb", bufs=2) as osb,
            tc.tile_pool(name="hps", bufs=1, space="PSUM") as hps,
            tc.tile_pool(name="ops", bufs=1, space="PSUM") as opsp,
            tc.tile_pool(name="g16ps", bufs=1, space="PSUM") as g16ps,
        ):
            w1_sb = singles.tile([KX, 4, 128], BF16)
            nc.sync.dma_start(w1_sb, w1.rearrange("k (w m) -> k w m", w=4))
            w1g_sb = singles.tile([KX, 32], BF16)
            nc.sync.dma_start(w1g_sb, w1g)
            w2_sb = singles.tile([128, 4, 32], BF16)
            nc.sync.dma_start(w2_sb, w2.rearrange("k (w m) -> k w m", w=4))
            w2g_sb = singles.tile([128, 32], BF16)
            nc.sync.dma_start(w2g_sb, w2g)
            b2v_sb = singles.tile([128, 1], F32)
            nc.sync.dma_start(b2v_sb, b2v)
            b2g_sb = singles.tile([32, 1], F32)
            nc.sync.dma_start(b2g_sb, b2g)

            nblocks = NFULL + 1
            h4acc = None
            xts_super = []
            out_t = None
            prev_l2 = None      # (hts, nb, off, blk) for L2 delayed a block
            pending_g16 = None  # (h4acc, scol, cols, rows)
            g16_mm = None       # carried between the two pending stages

            def do_l2(hts, nb, off, blk):
                """Delayed layer 2; returns output-evac work when a
                pair closes."""
                nonlocal out_t
                par = blk % 2
                if par == 0:
                    out_t = opsp.tile([128, 2, NBLK], F32, tag="out",
                                      name="out_t")
                for w in range(4):
                    nc.tensor.matmul(
                        out_t[32 * w:32 * w + 32, par, :nb],
                        lhsT=w2_sb[:, w, :],
                        rhs=hts[w][:, :nb],
                        start=True, stop=True,
                        tile_position=(0, 32 * w),
                    )
                if par == 1 or blk == NFULL:
                    ncols = NBLK + nb if par == 1 else nb
                    poff = off - NBLK if par == 1 else off
                    return (out_t, ncols, poff)
                return None

            for blk in range(nblocks):
                off = blk * NBLK
                nb = min(NBLK, S - off)
                b4 = blk % 4 if blk < NFULL else 0

                # pending group-16 flush, stage 1: free the h4acc bank
                # early in this block's VectorE queue
                if pending_g16 is not None:
                    ph4, pscol, pcols, prows = pending_g16
                    h4sb = hsb.tile([128, NBLK], BF16, tag="h4")
                    nc.vector.tensor_scalar(
                        h4sb[:prows, :pcols], ph4[:prows, :pcols],
                        0.0, None, MAX,
                    )
                    g16_mm = (h4sb, pscol, pcols)
                    pending_g16 = None

                xt = xin.tile([KX, NBLK], BF16, tag="xt")
                nc.sync.dma_start(xt[:, :nb], x2[:, off:off + nb])

                # ---- layer 1: 2-bank pair + two single banks ----
                # passes 2/3 (single banks, fast DVE evacs) run first;
                # the ACT-paired hp01 fills last so its longer evac
                # overlaps the next block's w2/w3 instead of gating w0.
                hp01 = hps.tile([128, 2, NBLK], F32, tag="hp01")
                hp2 = hps.tile([128, NBLK], F32, tag="hp2")
                hp3 = hps.tile([128, NBLK], F32, tag="hp3")
                nc.tensor.matmul(
                    hp2[:, :nb], lhsT=w1_sb[:, 2, :], rhs=xt[:, :nb],
                    start=True, stop=True,
                )
                nc.tensor.matmul(
                    hp3[:, :nb], lhsT=w1_sb[:, 3, :], rhs=xt[:, :nb],
                    start=True, stop=True,
                )
                for w in range(2):
                    nc.tensor.matmul(
                        hp01[:, w, :nb], lhsT=w1_sb[:, w, :], rhs=xt[:, :nb],
                        start=True, stop=True,
                    )
                if b4 == 0:
                    xts_super = []
                xts_super.append((xt, nb))

                # ---- delayed layer 2 of the previous block ----
                pending_osb = None
                if prev_l2 is not None:
                    pending_osb = do_l2(*prev_l2)

                # pending group-16 flush, stage 2: L2 matmul + evac +
                # store (h4sb evac from stage 1 is done by now)
                og16_out = None
                if g16_mm is not None:
                    gh4sb, gscol, gcols = g16_mm
                    og16 = g16ps.tile([32, NBLK], F32, tag="og16",
                                      name="og16")
                    nc.tensor.matmul(
                        og16[:, :gcols], lhsT=w2g_sb, rhs=gh4sb[:, :gcols],
                        start=True, stop=True,
                    )
                    og16_out = (og16, gscol, gcols)
                    g16_mm = None

                # ---- batched group-16 L1: 4 concurrent col strips ----
                if (b4 == 3) or blk == NFULL:
                    h4acc = g16ps.tile([128, NBLK], F32, tag="h4acc",
                                       name="h4acc")
                    for q, (qxt, qnb) in enumerate(xts_super):
                        nc.tensor.matmul(
                            h4acc[32 * q:32 * q + 32, :qnb],
                            lhsT=w1g_sb, rhs=qxt[:, :qnb],
                            start=True, stop=True,
                            tile_position=(0, 32 * q),
                        )

                # ---- relu evacuations (cast to bf16) ----
                h01 = hsb.tile([128, 2, NBLK], BF16, tag="h01")
                h2 = hsb.tile([128, NBLK], BF16, tag="h2")
                h3 = hsb.tile([128, NBLK], BF16, tag="h3")
                nc.vector.tensor_scalar(
                    h2[:, :nb], hp2[:, :nb], 0.0, None, MAX,
                )
                nc.vector.tensor_scalar(
                    h3[:, :nb], hp3[:, :nb], 0.0, None, MAX,
                )
                nc.scalar.activation(
                    out=h01[:, :, :nb], in_=hp01[:, :, :nb], func=RELU,
                )
                prev_l2 = ((h01[:, 0], h01[:, 1], h2, h3), nb, off, blk)

                # ---- deferred output stores (issued after the relu
                # evacs so they never delay them in the engine queues) ----
                if pending_osb is not None:
                    pout_t, ncols, poff = pending_osb
                    posb = osb.tile([128, 2, NBLK], BF16, tag="osb",
                                    name="posb")
                    osrc = pout_t[:120, :, :].rearrange("p a b -> p (a b)")
                    odst = posb[:120, :, :].rearrange("p a b -> p (a b)")
                    nc.scalar.activation(
                        out=odst[:, :ncols], in_=osrc[:, :ncols],
                        func=IDENT, bias=b2v_sb[:120], scale=1.0,
                    )
                    nc.gpsimd.dma_start(outd[:, poff:poff + ncols],
                                        odst[:, :ncols])
                if og16_out is not None:
                    gog16, gscol, gcols = og16_out
                    og16sb = osb.tile([32, NBLK], BF16, tag="og16sb",
                                      name="og16sb")
                    nc.vector.tensor_scalar(
                        og16sb[:, :gcols], gog16[:, :gcols], b2g_sb, None, ADD,
                    )
                    nc.gpsimd.dma_start(og16d[:, gscol:gscol + gcols],
                                        og16sb[:24, :gcols])

                if (b4 == 3) or blk == NFULL:
                    scol = (blk // 4) * NBLK if blk < NFULL else NSUPER * NBLK
                    cols = NBLK if blk < NFULL else TAILN
                    rows = 128 if blk < NFULL else 32
                    pending_g16 = (h4acc, scol, cols, rows)

            # ---- drain: final block's L2 + the tail's group-16 ----
            pending_osb = do_l2(*prev_l2)
            pout_t, ncols, poff = pending_osb
            posb = osb.tile([128, 2, NBLK], BF16, tag="osb", name="posb")
            osrc = pout_t[:120, :, :].rearrange("p a b -> p (a b)")
            odst = posb[:120, :, :].rearrange("p a b -> p (a b)")
            nc.scalar.activation(
                out=odst[:, :ncols], in_=osrc[:, :ncols],
                func=IDENT, bias=b2v_sb[:120], scale=1.0,
            )
            nc.gpsimd.dma_start(outd[:, poff:poff + ncols], odst[:, :ncols])
            ph4, pscol, pcols, prows = pending_g16
            h4sb = hsb.tile([128, NBLK], BF16, tag="h4")
            nc.vector.tensor_scalar(
                h4sb[:prows, :pcols], ph4[:prows, :pcols], 0.0, None, MAX,
            )
            og16 = g16ps.tile([32, NBLK], F32, tag="og16", name="og16")
            nc.tensor.matmul(
                og16[:, :pcols], lhsT=w2g_sb, rhs=h4sb[:, :pcols],
                start=True, stop=True,
            )
            og16sb = osb.tile([32, NBLK], BF16, tag="og16sb", name="og16sb")
            nc.vector.tensor_scalar(
                og16sb[:, :pcols], og16[:, :pcols], b2g_sb, None, ADD,
            )
            nc.gpsimd.dma_start(og16d[:, pscol:pscol + pcols],
                                og16sb[:24, :pcols])
    nc.finalize()
    return nc


_NC_CACHE = None


def _get_nc():
    global _NC_CACHE
    if _NC_CACHE is None:
        _NC_CACHE = _build_nc()
    return _NC_CACHE


def _kernel_impl(x, W1, b1, W2, b2, idx, _want_trace=False):
    x = np.asarray(x, np.float32)
    w1l, w1g, w2l, w2g, b2v, b2g = _host_weights(W1, b1, W2, b2, idx)

    in_maps = []
    for c in range(NCORES):
        xc = x[c * BC:(c + 1) * BC].reshape(TC, NF)
        xt2 = np.empty((KX, S), BF16_NP)
        xt2[0:NF] = np.ascontiguousarray(xc[:S].T)
        xt2[NF] = np.float32(1.0)
        xt2[NF + 1:2 * NF + 1] = np.ascontiguousarray(xc[S:].T)
        xt2[2 * NF + 1] = 0.0
        in_maps.append({
            "x2": xt2, "w1": w1l, "w1g": w1g, "w2": w2l, "w2g": w2g,
            "b2v": b2v, "b2g": b2g,
        })

    nc = _get_nc()
    res = run_bass_kernel_spmd(
        nc, in_maps, core_ids=list(range(NCORES)), trace=_want_trace,
    )

    # psum row of (group g<16, coord c): strip w=g//4, col 3*(g%4)+c
    ga = np.arange(16)
    rows_a = (32 * (ga // 4)[:, None] + 3 * (ga % 4)[:, None]
              + np.arange(3)).ravel()
    rows_b = rows_a + 12

    out = np.empty((B, T, NG, C), np.float32)
    for c in range(NCORES):
        od = res.results[c]["outd"].astype(np.float32)      # [120, S]
        og = res.results[c]["og16d"].astype(np.float32)     # [24, OG16_COLS]
        oc = np.empty((TC, NG * C), np.float32)
        oc[:S, :48] = od[rows_a].T
        oc[S:, :48] = od[rows_b].T
        # group 16: full supersteps [24, 15, 512] -> [s, b, j, 3set+c]
        full = og[:, :NSUPER * NBLK].reshape(4, 6, NSUPER, NBLK)
        tmp = full.transpose(2, 0, 3, 1).reshape(NSUPER * 4 * NBLK, 6)
        oc[:NFULL * NBLK, 48:51] = tmp[:, 0:3]
        oc[S:S + NFULL * NBLK, 48:51] = tmp[:, 3:6]
        tl = og[:6, NSUPER * NBLK:]                          # [6, TAILN]
        oc[NFULL * NBLK:S, 48:51] = tl[0:3].T
        oc[S + NFULL * NBLK:, 48:51] = tl[3:6].T
        out[c * BC:(c + 1) * BC] = oc.reshape(BC, T, NG, C)
    return out, res


def kernel(**inputs):
    out, _ = _kernel_impl(**inputs)
    return out
